# revision 25
# baseline (speedup 1.0000x reference)
"""GRU cell kernel for Trainium2, data-parallel over 8 NeuronCores.

Math (per reference):
    z = sigmoid(x @ wz.T + h @ uz.T + bz)
    r = sigmoid(x @ wr.T + h @ ur.T + br)
    g = tanh(x @ wh.T + (r*h) @ uh.T + bh)
    out = (1-z)*h + z*g

Everything on-device is computed in TRANSPOSED layout ([feature, row]),
so that both matmul operands arrive with the contraction dim on
partitions without any on-device transpose.

Precision/speed split (validated against the fp32 reference offline;
device rel-err matches the numpy sim to 5 digits):
  - r-gate matmuls and (r*h)@uh: full fp8-e4m3 DoubleRow (2 contraction
    rows per PE cell per cycle -> ~2x bf16 throughput).
  - x@wz: K-rows 0..767 bf16, 768..1023 fp8 DR.
  - h@uz: K-rows 0..255 bf16, 256..1023 fp8 DR (the uz side tolerates
    far more fp8 than wz at the max-err metric).
  - x@wh: K-rows 0..511 bf16, 512..1023 fp8 DR.
  Sim rel-err 0.0185 vs gate 2e-2 (device matches the sim exactly).
All moving operands are pre-scaled x16 and all weights x128 on host, so
every PSUM holds 2048*(pre-activation); the activation instruction
undoes it with scale=1/2048 before bias.  (1-z) comes from a second
sigmoid with negated scale/bias on the otherwise-idle Scalar engine, so
(1-z)*h is precomputed in the z pass and the post-matmul tail chain is
just tanh -> z*g -> add.

DMA-issue notes: descriptor issue (DIRECT2D) costs ~650ns serialized on
the Sync queue (and ~1.6us on Scalar, where it also blocks activations,
so everything stays on Sync).  The layout minimizes descriptor count on
the critical path: fp8 weights are host-packed so a pair-tile is one
descriptor, x8/h8 load two k-blocks per descriptor via an AP rearrange,
and emission follows consumption order (r set, z set, xt/hbt, h set).
Chunk-1 operand loads are emitted before chunk-0's h-pass stores so the
stores' data-dependency waits cannot delay them in the queue.

Sharding: rows 16384 -> 8 cores x 2048 rows, weights replicated and
loaded once (reused across both row-chunks).
"""

import numpy as np
import ml_dtypes
from contextlib import ExitStack

import concourse.bass as bass
import concourse.bacc as bacc
import concourse.mybir as mybir
import concourse.tile as tile
from concourse.bass_utils import run_bass_kernel_spmd

H = 1024
N_ROWS = 16384
NCORES = 8
P = 128
KB = H // P            # 8 contraction blocks (bf16)
KP = KB // 2           # 4 fp8 DoubleRow contraction pairs
MB = H // P            # 8 output-feature blocks
NS = 512               # rows per matmul moving slice (one PSUM bank)
KZ8X = 1               # x@wz: trailing DR pairs in fp8 (of KP)
KZ8H = 3               # h@uz: trailing DR pairs in fp8 (asymmetric: the
                       # uz side tolerates more fp8 than wz — sim-verified
                       # at rel-err 0.0185 vs the 2e-2 gate)
KH8 = 2                # x@wh: trailing DR pairs in fp8 (of KP)
KBZX = KB - 2 * KZ8X   # 6 bf16 k-blocks in x@wz
KBZH = KB - 2 * KZ8H   # 4 bf16 k-blocks in h@uz
KBH = KB - 2 * KH8     # 4 bf16 k-blocks in x@wh

BF = mybir.dt.bfloat16
F8 = mybir.dt.float8e4
F32 = mybir.dt.float32
AF = mybir.ActivationFunctionType
DR = mybir.MatmulPerfMode.DoubleRow
bf16 = ml_dtypes.bfloat16
f8e4 = ml_dtypes.float8_e4m3

SX = 16.0              # moving-operand scale
SW = 128.0             # weight scale
INV_S = 1.0 / (SX * SW)

# Set by test harness to capture a trace; harness-facing default off.
TRACE = False
LAST_RESULT = None


def build_nc(R=N_ROWS // NCORES, CH=2):
    """Build the per-core Bass program. R rows per core, CH row-chunks."""
    RC = R // CH           # rows per chunk
    SL = RC // NS          # moving slices per chunk

    nc = bacc.Bacc(trn_type="TRN2", target_bir_lowering=False,
                   debug=False, enable_asserts=False)

    xb = nc.dram_tensor("xb", [H, R], BF, kind="ExternalInput").ap()
    x8 = nc.dram_tensor("x8", [H, R], F8, kind="ExternalInput").ap()
    hb = nc.dram_tensor("hb", [H, R], BF, kind="ExternalInput").ap()
    h8 = nc.dram_tensor("h8", [H, R], F8, kind="ExternalInput").ap()
    hc = nc.dram_tensor("hc", [H, R], BF, kind="ExternalInput").ap()
    wd = {
        nm: nc.dram_tensor(nm, [H, H], BF, kind="ExternalInput").ap()
        for nm in ("wzT", "uzT", "whT")
    }
    # fp8 weights host-packed as [P, KP, 2, H] so one pair-tile is one
    # contiguous-per-partition DMA descriptor
    wd8 = {
        nm: nc.dram_tensor(nm, [P, KP * 2 * H], F8, kind="ExternalInput").ap()
        for nm in ("wzT8", "uzT8", "whT8", "wrT8", "urT8", "uhT8")
    }
    bias = nc.dram_tensor("bias", [P, 4 * MB], F32, kind="ExternalInput").ap()
    outT = nc.dram_tensor("outT", [H, R], F32, kind="ExternalOutput").ap()

    with tile.TileContext(nc) as tc, ExitStack() as ctx:
        wbpool = ctx.enter_context(tc.tile_pool(name="wb", bufs=16))
        w8pool = ctx.enter_context(tc.tile_pool(name="w8", bufs=18))
        xpool = ctx.enter_context(tc.tile_pool(name="x", bufs=2))
        x8pool = ctx.enter_context(tc.tile_pool(name="x8", bufs=2))
        hbpool = ctx.enter_context(tc.tile_pool(name="hb", bufs=1))
        h8pool = ctx.enter_context(tc.tile_pool(name="h8", bufs=1))
        hcpool = ctx.enter_context(tc.tile_pool(name="hc", bufs=2))
        ctpool = ctx.enter_context(tc.tile_pool(name="ct", bufs=MB + 1))
        rh8pool = ctx.enter_context(tc.tile_pool(name="rh8", bufs=2))
        rpool = ctx.enter_context(tc.tile_pool(name="r", bufs=4))
        zpool = ctx.enter_context(tc.tile_pool(name="z", bufs=2 * MB))
        gpool = ctx.enter_context(tc.tile_pool(name="g", bufs=3))
        opool = ctx.enter_context(tc.tile_pool(name="o", bufs=4))
        cpool = ctx.enter_context(tc.tile_pool(name="c", bufs=1))
        wfpool = ctx.enter_context(tc.tile_pool(name="wf", bufs=2))
        pspool = ctx.enter_context(tc.tile_pool(name="ps", bufs=8, space="PSUM"))

        # Warm up the ACT table set (sigmoid_and_others covers tanh too) on an
        # instruction with minimal sync waits — walrus can't attach the
        # PSEUDO_LOAD_ACT_FUNC_SET to an activation that already carries two
        # sem waits ("Too many sync wait commands").
        warm = cpool.tile([P, 8], F32, tag="warm")
        nc.gpsimd.memset(warm[:], 0.0)
        nc.scalar.activation(warm[:], warm[:], AF.Sigmoid)

        # bias column layout: [z:0..7 | r:8..15 | h:16..23 | -z:24..31]
        # (DMA emitted after the r set — first needed by the m0 sigmoid)
        bt = cpool.tile([P, 4 * MB], F32, tag="bias")
        GZ, GR, GH, GN = 0, 1, 2, 3

        def load_wb(name, nk):
            """nk leading bf16 k-tiles [P, H] of one weight matrix."""
            ts = []
            for k in range(nk):
                t = wbpool.tile([P, H], BF, tag="wb")
                nc.sync.dma_start(t[:], wd[name][k * P:(k + 1) * P, :])
                ts.append(t)
            return ts

        def load_w8(name, j0=0, j1=KP):
            """fp8 DR pair-tiles [P, 2, H], one descriptor each."""
            ts = []
            for j in range(j0, j1):
                t = w8pool.tile([P, 2, H], F8, tag="w8", name="t")
                nc.sync.dma_start(t[:, :, :],
                                  wd8[name][:, j * 2 * H:(j + 1) * 2 * H])
                ts.append(t)
            return ts

        def load_f8_ops(x8t, h8t, rows):
            """x8/h8 chunk loads, two k-blocks per descriptor."""
            for j in range(KP):
                ksl = slice(2 * j * P, (2 * j + 2) * P)
                nc.sync.dma_start(
                    x8t[:, 2 * j:2 * j + 2, :],
                    x8[ksl, rows].rearrange("(k p) c -> p k c", p=P))
                nc.sync.dma_start(
                    h8t[:, 2 * j:2 * j + 2, :],
                    h8[ksl, rows].rearrange("(k p) c -> p k c", p=P))

        def load_bf_ops(xt, hbt, rows):
            # hbt first: the r-pass epilogue (rh = r*16h) consumes hbt
            # m-block by m-block well before the z pass needs xt
            for k in range(KB):
                ksl = slice(k * P, (k + 1) * P)
                nc.sync.dma_start(hbt[:, k * RC:(k + 1) * RC], hb[ksl, rows])
            for k in range(KBZX):
                ksl = slice(k * P, (k + 1) * P)
                nc.sync.dma_start(xt[:, k * RC:(k + 1) * RC], xb[ksl, rows])

        # ---- chunk-0 r set (weights interleaved with operands), then z
        # set, then xt/hbt, then h set — matching consumption order ----
        x8t0 = x8pool.tile([P, KB, RC], F8, tag="x8", name="x8t")
        h8t0 = h8pool.tile([P, KB, RC], F8, tag="h8", name="h8t")
        wr, ur = [], []
        for j in range(KP):
            if j == 0:
                # duplicate 2D half-tiles let m0 start with two K=128
                # non-DR matmuls that each wait on only one 128KB weight
                # DMA and one x8 block; the pair tile (for m>=1) follows
                wr0h = []
                for k in range(2):
                    w0 = wfpool.tile([P, H], F8, tag="wf", name="w0")
                    nc.sync.dma_start(w0[:], wd8["wrT8"][:, k * H:(k + 1) * H])
                    nc.sync.dma_start(x8t0[:, k, :], x8[k * P:(k + 1) * P, 0:RC])
                    wr0h.append(w0)
                wr += load_w8("wrT8", 0, 1)
            else:
                wr += load_w8("wrT8", j, j + 1)
            ksl = slice(2 * j * P, (2 * j + 2) * P)
            if j != 0:
                nc.sync.dma_start(
                    x8t0[:, 2 * j:2 * j + 2, :],
                    x8[ksl, 0:RC].rearrange("(k p) c -> p k c", p=P))
            ur += load_w8("urT8", j, j + 1)
            if j == 0:
                for k in range(2):
                    ks1 = slice(k * P, (k + 1) * P)
                    nc.sync.dma_start(h8t0[:, k, :], h8[ks1, 0:RC])
            else:
                nc.sync.dma_start(
                    h8t0[:, 2 * j:2 * j + 2, :],
                    h8[ksl, 0:RC].rearrange("(k p) c -> p k c", p=P))
        nc.sync.dma_start(bt[:], bias[:])

        # hbt before the z weights: the r-pass epilogue (rh = r*16h)
        # consumes hbt m-block by m-block starting ~halfway into the r pass
        hbt0 = hbpool.tile([P, KB * RC], BF, tag="hb")
        for k in range(KB):
            ksl = slice(k * P, (k + 1) * P)
            nc.sync.dma_start(hbt0[:, k * RC:(k + 1) * RC], hb[ksl, 0:RC])

        wz = load_wb("wzT", KBZX)
        wz8 = load_w8("wzT8", KP - KZ8X, KP)
        uz = load_wb("uzT", KBZH)
        uz8 = load_w8("uzT8", KP - KZ8H, KP)

        xt0 = xpool.tile([P, KBZX * RC], BF, tag="x")
        for k in range(KBZX):
            ksl = slice(k * P, (k + 1) * P)
            nc.sync.dma_start(xt0[:, k * RC:(k + 1) * RC], xb[ksl, 0:RC])

        wh = load_wb("whT", KBH)
        wh8 = load_w8("whT8", KP - KH8, KP)
        uh = load_w8("uhT8")

        for c in range(CH):
            rows = slice(c * RC, (c + 1) * RC)

            if c == 0:
                x8t, h8t, xt, hbt = x8t0, h8t0, xt0, hbt0

            # ---- r pass (all fp8 DoubleRow) ----
            rh8 = rh8pool.tile([P, MB, RC], F8, tag="rh8")
            for m in range(MB):
                msl = slice(m * P, (m + 1) * P)
                ps = [pspool.tile([P, NS], F32, tag="ps", name="ps") for _ in range(SL)]
                for j in range(KP):
                    if c == 0 and m == 0 and j == 0:
                        for k in range(2):
                            for s in range(SL):
                                nc.tensor.matmul(
                                    ps[s][:], wr0h[k][:, msl],
                                    x8t[:, k, s * NS:(s + 1) * NS],
                                    start=(k == 0), stop=False)
                    else:
                        for s in range(SL):
                            nc.tensor.matmul(
                                ps[s][:], wr[j][:, :, msl],
                                x8t[:, 2 * j:2 * j + 2, s * NS:(s + 1) * NS],
                                start=(j == 0), stop=False, perf_mode=DR)
                    for s in range(SL):
                        nc.tensor.matmul(
                            ps[s][:], ur[j][:, :, msl],
                            h8t[:, 2 * j:2 * j + 2, s * NS:(s + 1) * NS],
                            start=False, stop=(j == KP - 1), perf_mode=DR)
                for s in range(SL):
                    rt = rpool.tile([P, NS], BF, tag="r")
                    nc.scalar.activation(rt[:], ps[s][:], AF.Sigmoid,
                                         bias=bt[:, GR * MB + m: GR * MB + m + 1],
                                         scale=INV_S)
                    # rh8 = e4m3(r * 16h): hbt is 16h, rt unscaled in (0,1)
                    nc.vector.tensor_mul(
                        rh8[:, m, s * NS:(s + 1) * NS], rt[:],
                        hbt[:, m * RC + s * NS: m * RC + (s + 1) * NS])

            # ---- z pass (bf16 k0..5 + fp8 DR pair k6,7) ----
            # also computes ct = (1-z)*h via a second sigmoid with negated
            # scale/bias, so the h~ pass tail is just tanh -> z*g -> +ct
            zts, cts = [], []
            for m in range(MB):
                msl = slice(m * P, (m + 1) * P)
                hct = hcpool.tile([P, RC], BF, tag="hc")
                nc.sync.dma_start(hct[:], hc[msl, rows])
                ps = [pspool.tile([P, NS], F32, tag="ps", name="ps") for _ in range(SL)]
                for k in range(KBZX):
                    for s in range(SL):
                        nc.tensor.matmul(
                            ps[s][:], wz[k][:, msl],
                            xt[:, k * RC + s * NS: k * RC + (s + 1) * NS],
                            start=(k == 0), stop=False)
                for jj, j in enumerate(range(KP - KZ8X, KP)):
                    for s in range(SL):
                        nc.tensor.matmul(
                            ps[s][:], wz8[jj][:, :, msl],
                            x8t[:, 2 * j:2 * j + 2, s * NS:(s + 1) * NS],
                            start=False, stop=False, perf_mode=DR)
                for k in range(KBZH):
                    for s in range(SL):
                        nc.tensor.matmul(
                            ps[s][:], uz[k][:, msl],
                            hbt[:, k * RC + s * NS: k * RC + (s + 1) * NS],
                            start=False, stop=False)
                for jj, j in enumerate(range(KP - KZ8H, KP)):
                    for s in range(SL):
                        nc.tensor.matmul(
                            ps[s][:], uz8[jj][:, :, msl],
                            h8t[:, 2 * j:2 * j + 2, s * NS:(s + 1) * NS],
                            start=False, stop=(j == KP - 1), perf_mode=DR)
                zm = []
                ctt = ctpool.tile([P, RC], BF, tag="ct")
                for s in range(SL):
                    ssl = slice(s * NS, (s + 1) * NS)
                    zt = zpool.tile([P, NS], BF, tag="z")
                    nc.scalar.activation(zt[:], ps[s][:], AF.Sigmoid,
                                         bias=bt[:, GZ * MB + m: GZ * MB + m + 1],
                                         scale=INV_S)
                    zm.append(zt)
                    zct = rpool.tile([P, NS], BF, tag="r")
                    nc.scalar.activation(zct[:], ps[s][:], AF.Sigmoid,
                                         bias=bt[:, GN * MB + m: GN * MB + m + 1],
                                         scale=-INV_S)
                    nc.vector.tensor_mul(ctt[:, ssl], zct[:], hct[:, ssl])
                zts.append(zm)
                cts.append(ctt)

            # chunk c+1 operand loads, emitted BEFORE this chunk's h-pass
            # stores so the stores' data waits can't delay them in the
            # Sync queue
            if c + 1 < CH:
                nrows = slice((c + 1) * RC, (c + 2) * RC)
                nx8t = x8pool.tile([P, KB, RC], F8, tag="x8", name="x8t")
                nh8t = h8pool.tile([P, KB, RC], F8, tag="h8", name="h8t")
                load_f8_ops(nx8t, nh8t, nrows)
                nxt = xpool.tile([P, KBZX * RC], BF, tag="x")
                nhbt = hbpool.tile([P, KB * RC], BF, tag="hb")
                load_bf_ops(nxt, nhbt, nrows)

            # ---- h~ pass (x@wh bf16 k0..3 + fp8 DR pairs; (r*h)@uh fp8 DR)
            #      + combine ----
            for m in range(MB):
                msl = slice(m * P, (m + 1) * P)
                ps = [pspool.tile([P, NS], F32, tag="ps", name="ps") for _ in range(SL)]
                for k in range(KBH):
                    for s in range(SL):
                        nc.tensor.matmul(
                            ps[s][:], wh[k][:, msl],
                            xt[:, k * RC + s * NS: k * RC + (s + 1) * NS],
                            start=(k == 0), stop=False)
                for jj, j in enumerate(range(KP - KH8, KP)):
                    for s in range(SL):
                        nc.tensor.matmul(
                            ps[s][:], wh8[jj][:, :, msl],
                            x8t[:, 2 * j:2 * j + 2, s * NS:(s + 1) * NS],
                            start=False, stop=False, perf_mode=DR)
                for j in range(KP):
                    for s in range(SL):
                        nc.tensor.matmul(
                            ps[s][:], uh[j][:, :, msl],
                            rh8[:, 2 * j:2 * j + 2, s * NS:(s + 1) * NS],
                            start=False, stop=(j == KP - 1), perf_mode=DR)
                # the very last m-block runs its epilogue at 256-col
                # granularity so the post-matmul drain pipeline is shorter
                fine = (c == CH - 1 and m == MB - 1)
                NE = NS // 2 if fine else NS
                for s in range(SL):
                    for e in range(NS // NE):
                        esl = slice(s * NS + e * NE, s * NS + (e + 1) * NE)
                        pesl = slice(e * NE, (e + 1) * NE)
                        gt = gpool.tile([P, NE], F32, tag="g")
                        nc.scalar.activation(gt[:], ps[s][:, pesl], AF.Tanh,
                                             bias=bt[:, GH * MB + m:
                                                     GH * MB + m + 1],
                                             scale=INV_S)
                        # z*g ; (1-z)*h + z*g
                        nc.vector.tensor_mul(gt[:], zts[m][s][:, pesl], gt[:])
                        ot = opool.tile([P, NE], F32, tag="o")
                        nc.vector.tensor_add(ot[:], gt[:], cts[m][:, esl])
                        # one store descriptor per piece: descriptor issue
                        # (~650ns, serialized on Sync) costs more than the
                        # extra per-ring transfer time
                        nc.sync.dma_start(outT[msl, c * RC + s * NS + e * NE:
                                               c * RC + s * NS + (e + 1) * NE],
                                          ot[:])

            if c + 1 < CH:
                x8t, h8t, xt, hbt = nx8t, nh8t, nxt, nhbt

    nc.compile()
    return nc


_NC_CACHE = {}


def _get_nc(R, CH):
    key = (R, CH)
    if key not in _NC_CACHE:
        _NC_CACHE[key] = build_nc(R, CH)
    return _NC_CACHE[key]


def make_in_maps(update, hidden, wz, uz, bz, wr, ur, br, wh, uh, bh,
                 ncores=NCORES):
    wmap = {}
    for nm, w in (("wzT", wz), ("uzT", uz), ("whT", wh)):
        ws = np.ascontiguousarray(np.asarray(w, np.float32).T * SW)
        wmap[nm] = ws.astype(bf16)
    # fp8 weights packed [P, KP, 2, H]: pack[p, j, i, m] = W.T[(2j+i)*P+p, m]
    for nm, w in (("wzT8", wz), ("uzT8", uz), ("whT8", wh),
                  ("wrT8", wr), ("urT8", ur), ("uhT8", uh)):
        ws = (np.asarray(w, np.float32).T * SW).astype(f8e4)
        pk = ws.reshape(KP, 2, P, H).transpose(2, 0, 1, 3).reshape(P, KP * 2 * H)
        wmap[nm] = np.ascontiguousarray(pk)
    bias = np.empty((P, 4 * MB), np.float32)
    for g, b in enumerate((bz, br, bh)):
        bias[:, g * MB:(g + 1) * MB] = np.asarray(b, np.float32).reshape(MB, P).T
    bias[:, 3 * MB:4 * MB] = -bias[:, 0:MB]
    rows = update.shape[0]
    rc = rows // ncores
    in_maps = []
    for i in range(ncores):
        sl = slice(i * rc, (i + 1) * rc)
        xTs = np.ascontiguousarray(np.asarray(update[sl], np.float32).T)
        hTs = np.ascontiguousarray(np.asarray(hidden[sl], np.float32).T)
        x16 = xTs * SX
        h16 = hTs * SX
        in_maps.append(dict(
            xb=x16.astype(bf16), x8=x16.astype(f8e4),
            hb=h16.astype(bf16), h8=h16.astype(f8e4),
            hc=hTs.astype(bf16), bias=bias, **wmap))
    return in_maps


def kernel(update, hidden, wz, uz, bz, wr, ur, br, wh, uh, bh):
    global LAST_RESULT
    update = np.asarray(update)
    hidden = np.asarray(hidden)
    R = update.shape[0] // NCORES
    nc = _get_nc(R, 2)
    in_maps = make_in_maps(update, hidden, wz, uz, bz, wr, ur, br, wh, uh, bh)
    res = run_bass_kernel_spmd(nc, in_maps, list(range(NCORES)), trace=TRACE)
    LAST_RESULT = res
    out = np.empty((update.shape[0], H), np.float32)
    for i in range(NCORES):
        out[i * R:(i + 1) * R] = res.results[i]["outT"].T
    return out


# revision 26
# speedup vs baseline: 1.0137x; 1.0137x over previous
"""GRU cell kernel for Trainium2, data-parallel over 8 NeuronCores.

Math (per reference):
    z = sigmoid(x @ wz.T + h @ uz.T + bz)
    r = sigmoid(x @ wr.T + h @ ur.T + br)
    g = tanh(x @ wh.T + (r*h) @ uh.T + bh)
    out = (1-z)*h + z*g

Everything on-device is computed in TRANSPOSED layout ([feature, row]),
so that both matmul operands arrive with the contraction dim on
partitions without any on-device transpose.

Precision/speed split (validated against the fp32 reference offline;
device rel-err matches the numpy sim to 5 digits):
  - r-gate matmuls and (r*h)@uh: full fp8-e4m3 DoubleRow (2 contraction
    rows per PE cell per cycle -> ~2x bf16 throughput).
  - x@wz: K-rows 0..767 bf16, 768..1023 fp8 DR.
  - h@uz: K-rows 0..255 bf16, 256..1023 fp8 DR (the uz side tolerates
    far more fp8 than wz at the max-err metric).
  - x@wh: K-rows 0..511 bf16, 512..1023 fp8 DR.
  Sim rel-err 0.0185 vs gate 2e-2 (device matches the sim exactly).
All moving operands are pre-scaled x16 and all weights x128 on host, so
every PSUM holds 2048*(pre-activation); the activation instruction
undoes it with scale=1/2048 before bias.  (1-z) comes from a second
sigmoid with negated scale/bias on the otherwise-idle Scalar engine, so
(1-z)*h is precomputed in the z pass and the post-matmul tail chain is
just tanh -> z*g -> add.

DMA-issue notes: descriptor issue (DIRECT2D) costs ~650ns serialized on
the Sync queue (and ~1.6us on Scalar, where it also blocks activations,
so everything stays on Sync).  The layout minimizes descriptor count on
the critical path: fp8 weights are host-packed so a pair-tile is one
descriptor, x8/h8 load two k-blocks per descriptor via an AP rearrange,
and emission follows consumption order (r set, z set, xt/hbt, h set).
Chunk-1 operand loads are emitted before chunk-0's h-pass stores so the
stores' data-dependency waits cannot delay them in the queue.

Sharding: rows 16384 -> 8 cores x 2048 rows, weights replicated and
loaded once (reused across both row-chunks).
"""

import numpy as np
import ml_dtypes
from contextlib import ExitStack

import concourse.bass as bass
import concourse.bacc as bacc
import concourse.mybir as mybir
import concourse.tile as tile
from concourse.bass_utils import run_bass_kernel_spmd

H = 1024
N_ROWS = 16384
NCORES = 8
P = 128
KB = H // P            # 8 contraction blocks (bf16)
KP = KB // 2           # 4 fp8 DoubleRow contraction pairs
MB = H // P            # 8 output-feature blocks
NS = 512               # rows per matmul moving slice (one PSUM bank)
KZ8X = 1               # x@wz: trailing DR pairs in fp8 (of KP)
KZ8H = 3               # h@uz: trailing DR pairs in fp8 (asymmetric: the
                       # uz side tolerates more fp8 than wz — sim-verified
                       # at rel-err 0.0185 vs the 2e-2 gate)
KH8 = 2                # x@wh: trailing DR pairs in fp8 (of KP)
KBZX = KB - 2 * KZ8X   # 6 bf16 k-blocks in x@wz
KBZH = KB - 2 * KZ8H   # 4 bf16 k-blocks in h@uz
KBH = KB - 2 * KH8     # 4 bf16 k-blocks in x@wh

BF = mybir.dt.bfloat16
F8 = mybir.dt.float8e4
F32 = mybir.dt.float32
AF = mybir.ActivationFunctionType
DR = mybir.MatmulPerfMode.DoubleRow
bf16 = ml_dtypes.bfloat16
f8e4 = ml_dtypes.float8_e4m3

SX = 16.0              # moving-operand scale
SW = 128.0             # weight scale
INV_S = 1.0 / (SX * SW)

# Set by test harness to capture a trace; harness-facing default off.
TRACE = False
LAST_RESULT = None


def build_nc(R=N_ROWS // NCORES, CH=2):
    """Build the per-core Bass program. R rows per core, CH row-chunks."""
    RC = R // CH           # rows per chunk
    SL = RC // NS          # moving slices per chunk

    nc = bacc.Bacc(trn_type="TRN2", target_bir_lowering=False,
                   debug=False, enable_asserts=False)

    xb = nc.dram_tensor("xb", [H, R], BF, kind="ExternalInput").ap()
    x8 = nc.dram_tensor("x8", [H, R], F8, kind="ExternalInput").ap()
    hb = nc.dram_tensor("hb", [H, R], BF, kind="ExternalInput").ap()
    h8 = nc.dram_tensor("h8", [H, R], F8, kind="ExternalInput").ap()
    hc = nc.dram_tensor("hc", [H, R], BF, kind="ExternalInput").ap()
    wd = {
        nm: nc.dram_tensor(nm, [H, H], BF, kind="ExternalInput").ap()
        for nm in ("wzT", "uzT", "whT")
    }
    # fp8 weights host-packed as [P, KP, 2, H] so one pair-tile is one
    # contiguous-per-partition DMA descriptor
    wd8 = {
        nm: nc.dram_tensor(nm, [P, KP * 2 * H], F8, kind="ExternalInput").ap()
        for nm in ("wzT8", "uzT8", "whT8", "wrT8", "urT8", "uhT8")
    }
    bias = nc.dram_tensor("bias", [P, 4 * MB], F32, kind="ExternalInput").ap()
    outT = nc.dram_tensor("outT", [H, R], F32, kind="ExternalOutput").ap()

    with tile.TileContext(nc) as tc, ExitStack() as ctx:
        wbpool = ctx.enter_context(tc.tile_pool(name="wb", bufs=16))
        w8pool = ctx.enter_context(tc.tile_pool(name="w8", bufs=18))
        xpool = ctx.enter_context(tc.tile_pool(name="x", bufs=2))
        x8pool = ctx.enter_context(tc.tile_pool(name="x8", bufs=2))
        hbpool = ctx.enter_context(tc.tile_pool(name="hb", bufs=1))
        h8pool = ctx.enter_context(tc.tile_pool(name="h8", bufs=1))
        hcpool = ctx.enter_context(tc.tile_pool(name="hc", bufs=2))
        ctpool = ctx.enter_context(tc.tile_pool(name="ct", bufs=MB + 1))
        rh8pool = ctx.enter_context(tc.tile_pool(name="rh8", bufs=2))
        rpool = ctx.enter_context(tc.tile_pool(name="r", bufs=4))
        zpool = ctx.enter_context(tc.tile_pool(name="z", bufs=2 * MB))
        gpool = ctx.enter_context(tc.tile_pool(name="g", bufs=3))
        opool = ctx.enter_context(tc.tile_pool(name="o", bufs=4))
        cpool = ctx.enter_context(tc.tile_pool(name="c", bufs=1))
        pspool = ctx.enter_context(tc.tile_pool(name="ps", bufs=8, space="PSUM"))

        # Warm up the ACT table set (sigmoid_and_others covers tanh too) on an
        # instruction with minimal sync waits — walrus can't attach the
        # PSEUDO_LOAD_ACT_FUNC_SET to an activation that already carries two
        # sem waits ("Too many sync wait commands").
        warm = cpool.tile([P, 8], F32, tag="warm")
        nc.gpsimd.memset(warm[:], 0.0)
        nc.scalar.activation(warm[:], warm[:], AF.Sigmoid)

        # bias column layout: [z:0..7 | r:8..15 | h:16..23 | -z:24..31]
        # (DMA emitted after the r set — first needed by the m0 sigmoid)
        bt = cpool.tile([P, 4 * MB], F32, tag="bias")
        GZ, GR, GH, GN = 0, 1, 2, 3

        def load_wb(name, nk):
            """nk leading bf16 k-tiles [P, H] of one weight matrix."""
            ts = []
            for k in range(nk):
                t = wbpool.tile([P, H], BF, tag="wb")
                nc.sync.dma_start(t[:], wd[name][k * P:(k + 1) * P, :])
                ts.append(t)
            return ts

        def load_w8(name, j0=0, j1=KP):
            """fp8 DR pair-tiles [P, 2, H], one descriptor each."""
            ts = []
            for j in range(j0, j1):
                t = w8pool.tile([P, 2, H], F8, tag="w8", name="t")
                nc.sync.dma_start(t[:, :, :],
                                  wd8[name][:, j * 2 * H:(j + 1) * 2 * H])
                ts.append(t)
            return ts

        def load_f8_ops(x8t, h8t, rows):
            """x8/h8 chunk loads, two k-blocks per descriptor."""
            for j in range(KP):
                ksl = slice(2 * j * P, (2 * j + 2) * P)
                nc.sync.dma_start(
                    x8t[:, 2 * j:2 * j + 2, :],
                    x8[ksl, rows].rearrange("(k p) c -> p k c", p=P))
                nc.sync.dma_start(
                    h8t[:, 2 * j:2 * j + 2, :],
                    h8[ksl, rows].rearrange("(k p) c -> p k c", p=P))

        def load_bf_ops(xt, hbt, rows):
            # hbt first: the r-pass epilogue (rh = r*16h) consumes hbt
            # m-block by m-block well before the z pass needs xt
            for k in range(KB):
                ksl = slice(k * P, (k + 1) * P)
                nc.sync.dma_start(hbt[:, k * RC:(k + 1) * RC], hb[ksl, rows])
            for k in range(KBZX):
                ksl = slice(k * P, (k + 1) * P)
                nc.sync.dma_start(xt[:, k * RC:(k + 1) * RC], xb[ksl, rows])

        # ---- chunk-0 r set (weights interleaved with operands), then z
        # set, then xt/hbt, then h set — matching consumption order ----
        x8t0 = x8pool.tile([P, KB, RC], F8, tag="x8", name="x8t")
        h8t0 = h8pool.tile([P, KB, RC], F8, tag="h8", name="h8t")
        wr, ur = [], []
        for j in range(KP):
            if j == 0:
                # first tiles split in half so the very first matmul's
                # dependencies land sooner
                t = w8pool.tile([P, 2, H], F8, tag="w8", name="t")
                nc.sync.dma_start(t[:, 0, :], wd8["wrT8"][:, 0:H])
                nc.sync.dma_start(x8t0[:, 0, :], x8[0:P, 0:RC])
                nc.sync.dma_start(t[:, 1, :], wd8["wrT8"][:, H:2 * H])
                nc.sync.dma_start(x8t0[:, 1, :], x8[P:2 * P, 0:RC])
                wr.append(t)
            else:
                wr += load_w8("wrT8", j, j + 1)
            ksl = slice(2 * j * P, (2 * j + 2) * P)
            if j != 0:
                nc.sync.dma_start(
                    x8t0[:, 2 * j:2 * j + 2, :],
                    x8[ksl, 0:RC].rearrange("(k p) c -> p k c", p=P))
            ur += load_w8("urT8", j, j + 1)
            if j == 0:
                for k in range(2):
                    ks1 = slice(k * P, (k + 1) * P)
                    nc.sync.dma_start(h8t0[:, k, :], h8[ks1, 0:RC])
            else:
                nc.sync.dma_start(
                    h8t0[:, 2 * j:2 * j + 2, :],
                    h8[ksl, 0:RC].rearrange("(k p) c -> p k c", p=P))
        nc.sync.dma_start(bt[:], bias[:])

        # hbt before the z weights: the r-pass epilogue (rh = r*16h)
        # consumes hbt m-block by m-block starting ~halfway into the r pass
        hbt0 = hbpool.tile([P, KB * RC], BF, tag="hb")
        for k in range(KB):
            ksl = slice(k * P, (k + 1) * P)
            nc.sync.dma_start(hbt0[:, k * RC:(k + 1) * RC], hb[ksl, 0:RC])

        wz = load_wb("wzT", KBZX)
        wz8 = load_w8("wzT8", KP - KZ8X, KP)
        uz = load_wb("uzT", KBZH)
        uz8 = load_w8("uzT8", KP - KZ8H, KP)

        xt0 = xpool.tile([P, KBZX * RC], BF, tag="x")
        for k in range(KBZX):
            ksl = slice(k * P, (k + 1) * P)
            nc.sync.dma_start(xt0[:, k * RC:(k + 1) * RC], xb[ksl, 0:RC])

        wh = load_wb("whT", KBH)
        wh8 = load_w8("whT8", KP - KH8, KP)
        uh = load_w8("uhT8")

        for c in range(CH):
            rows = slice(c * RC, (c + 1) * RC)

            if c == 0:
                x8t, h8t, xt, hbt = x8t0, h8t0, xt0, hbt0

            # ---- r pass (all fp8 DoubleRow) ----
            rh8 = rh8pool.tile([P, MB, RC], F8, tag="rh8")
            for m in range(MB):
                msl = slice(m * P, (m + 1) * P)
                ps = [pspool.tile([P, NS], F32, tag="ps", name="ps") for _ in range(SL)]
                for j in range(KP):
                    for s in range(SL):
                        nc.tensor.matmul(
                            ps[s][:], wr[j][:, :, msl],
                            x8t[:, 2 * j:2 * j + 2, s * NS:(s + 1) * NS],
                            start=(j == 0), stop=False, perf_mode=DR)
                    for s in range(SL):
                        nc.tensor.matmul(
                            ps[s][:], ur[j][:, :, msl],
                            h8t[:, 2 * j:2 * j + 2, s * NS:(s + 1) * NS],
                            start=False, stop=(j == KP - 1), perf_mode=DR)
                for s in range(SL):
                    rt = rpool.tile([P, NS], BF, tag="r")
                    nc.scalar.activation(rt[:], ps[s][:], AF.Sigmoid,
                                         bias=bt[:, GR * MB + m: GR * MB + m + 1],
                                         scale=INV_S)
                    # rh8 = e4m3(r * 16h): hbt is 16h, rt unscaled in (0,1)
                    nc.vector.tensor_mul(
                        rh8[:, m, s * NS:(s + 1) * NS], rt[:],
                        hbt[:, m * RC + s * NS: m * RC + (s + 1) * NS])

            # ---- z pass (bf16 k0..5 + fp8 DR pair k6,7) ----
            # also computes ct = (1-z)*h via a second sigmoid with negated
            # scale/bias, so the h~ pass tail is just tanh -> z*g -> +ct
            zts, cts = [], []
            for m in range(MB):
                msl = slice(m * P, (m + 1) * P)
                hct = hcpool.tile([P, RC], BF, tag="hc")
                nc.sync.dma_start(hct[:], hc[msl, rows])
                ps = [pspool.tile([P, NS], F32, tag="ps", name="ps") for _ in range(SL)]
                for k in range(KBZX):
                    for s in range(SL):
                        nc.tensor.matmul(
                            ps[s][:], wz[k][:, msl],
                            xt[:, k * RC + s * NS: k * RC + (s + 1) * NS],
                            start=(k == 0), stop=False)
                for jj, j in enumerate(range(KP - KZ8X, KP)):
                    for s in range(SL):
                        nc.tensor.matmul(
                            ps[s][:], wz8[jj][:, :, msl],
                            x8t[:, 2 * j:2 * j + 2, s * NS:(s + 1) * NS],
                            start=False, stop=False, perf_mode=DR)
                for k in range(KBZH):
                    for s in range(SL):
                        nc.tensor.matmul(
                            ps[s][:], uz[k][:, msl],
                            hbt[:, k * RC + s * NS: k * RC + (s + 1) * NS],
                            start=False, stop=False)
                for jj, j in enumerate(range(KP - KZ8H, KP)):
                    for s in range(SL):
                        nc.tensor.matmul(
                            ps[s][:], uz8[jj][:, :, msl],
                            h8t[:, 2 * j:2 * j + 2, s * NS:(s + 1) * NS],
                            start=False, stop=(j == KP - 1), perf_mode=DR)
                zm = []
                ctt = ctpool.tile([P, RC], BF, tag="ct")
                for s in range(SL):
                    ssl = slice(s * NS, (s + 1) * NS)
                    zt = zpool.tile([P, NS], BF, tag="z")
                    nc.scalar.activation(zt[:], ps[s][:], AF.Sigmoid,
                                         bias=bt[:, GZ * MB + m: GZ * MB + m + 1],
                                         scale=INV_S)
                    zm.append(zt)
                    zct = rpool.tile([P, NS], BF, tag="r")
                    nc.scalar.activation(zct[:], ps[s][:], AF.Sigmoid,
                                         bias=bt[:, GN * MB + m: GN * MB + m + 1],
                                         scale=-INV_S)
                    nc.vector.tensor_mul(ctt[:, ssl], zct[:], hct[:, ssl])
                zts.append(zm)
                cts.append(ctt)

            # chunk c+1 operand loads, emitted BEFORE this chunk's h-pass
            # stores so the stores' data waits can't delay them in the
            # Sync queue
            if c + 1 < CH:
                nrows = slice((c + 1) * RC, (c + 2) * RC)
                nx8t = x8pool.tile([P, KB, RC], F8, tag="x8", name="x8t")
                nh8t = h8pool.tile([P, KB, RC], F8, tag="h8", name="h8t")
                load_f8_ops(nx8t, nh8t, nrows)
                nxt = xpool.tile([P, KBZX * RC], BF, tag="x")
                nhbt = hbpool.tile([P, KB * RC], BF, tag="hb")
                load_bf_ops(nxt, nhbt, nrows)

            # ---- h~ pass (x@wh bf16 k0..3 + fp8 DR pairs; (r*h)@uh fp8 DR)
            #      + combine ----
            for m in range(MB):
                msl = slice(m * P, (m + 1) * P)
                ps = [pspool.tile([P, NS], F32, tag="ps", name="ps") for _ in range(SL)]
                for k in range(KBH):
                    for s in range(SL):
                        nc.tensor.matmul(
                            ps[s][:], wh[k][:, msl],
                            xt[:, k * RC + s * NS: k * RC + (s + 1) * NS],
                            start=(k == 0), stop=False)
                for jj, j in enumerate(range(KP - KH8, KP)):
                    for s in range(SL):
                        nc.tensor.matmul(
                            ps[s][:], wh8[jj][:, :, msl],
                            x8t[:, 2 * j:2 * j + 2, s * NS:(s + 1) * NS],
                            start=False, stop=False, perf_mode=DR)
                for j in range(KP):
                    for s in range(SL):
                        nc.tensor.matmul(
                            ps[s][:], uh[j][:, :, msl],
                            rh8[:, 2 * j:2 * j + 2, s * NS:(s + 1) * NS],
                            start=False, stop=(j == KP - 1), perf_mode=DR)
                # the very last m-block runs its epilogue at 256-col
                # granularity so the post-matmul drain pipeline is shorter
                fine = (c == CH - 1 and m == MB - 1)
                NE = NS // 2 if fine else NS
                for s in range(SL):
                    for e in range(NS // NE):
                        esl = slice(s * NS + e * NE, s * NS + (e + 1) * NE)
                        pesl = slice(e * NE, (e + 1) * NE)
                        gt = gpool.tile([P, NE], F32, tag="g")
                        nc.scalar.activation(gt[:], ps[s][:, pesl], AF.Tanh,
                                             bias=bt[:, GH * MB + m:
                                                     GH * MB + m + 1],
                                             scale=INV_S)
                        # z*g ; (1-z)*h + z*g
                        nc.vector.tensor_mul(gt[:], zts[m][s][:, pesl], gt[:])
                        ot = opool.tile([P, NE], F32, tag="o")
                        nc.vector.tensor_add(ot[:], gt[:], cts[m][:, esl])
                        # one store descriptor per piece: descriptor issue
                        # (~650ns, serialized on Sync) costs more than the
                        # extra per-ring transfer time
                        nc.sync.dma_start(outT[msl, c * RC + s * NS + e * NE:
                                               c * RC + s * NS + (e + 1) * NE],
                                          ot[:])

            if c + 1 < CH:
                x8t, h8t, xt, hbt = nx8t, nh8t, nxt, nhbt

    nc.compile()
    return nc


_NC_CACHE = {}


def _get_nc(R, CH):
    key = (R, CH)
    if key not in _NC_CACHE:
        _NC_CACHE[key] = build_nc(R, CH)
    return _NC_CACHE[key]


def make_in_maps(update, hidden, wz, uz, bz, wr, ur, br, wh, uh, bh,
                 ncores=NCORES):
    wmap = {}
    for nm, w in (("wzT", wz), ("uzT", uz), ("whT", wh)):
        ws = np.ascontiguousarray(np.asarray(w, np.float32).T * SW)
        wmap[nm] = ws.astype(bf16)
    # fp8 weights packed [P, KP, 2, H]: pack[p, j, i, m] = W.T[(2j+i)*P+p, m]
    for nm, w in (("wzT8", wz), ("uzT8", uz), ("whT8", wh),
                  ("wrT8", wr), ("urT8", ur), ("uhT8", uh)):
        ws = (np.asarray(w, np.float32).T * SW).astype(f8e4)
        pk = ws.reshape(KP, 2, P, H).transpose(2, 0, 1, 3).reshape(P, KP * 2 * H)
        wmap[nm] = np.ascontiguousarray(pk)
    bias = np.empty((P, 4 * MB), np.float32)
    for g, b in enumerate((bz, br, bh)):
        bias[:, g * MB:(g + 1) * MB] = np.asarray(b, np.float32).reshape(MB, P).T
    bias[:, 3 * MB:4 * MB] = -bias[:, 0:MB]
    rows = update.shape[0]
    rc = rows // ncores
    in_maps = []
    for i in range(ncores):
        sl = slice(i * rc, (i + 1) * rc)
        xTs = np.ascontiguousarray(np.asarray(update[sl], np.float32).T)
        hTs = np.ascontiguousarray(np.asarray(hidden[sl], np.float32).T)
        x16 = xTs * SX
        h16 = hTs * SX
        in_maps.append(dict(
            xb=x16.astype(bf16), x8=x16.astype(f8e4),
            hb=h16.astype(bf16), h8=h16.astype(f8e4),
            hc=hTs.astype(bf16), bias=bias, **wmap))
    return in_maps


def kernel(update, hidden, wz, uz, bz, wr, ur, br, wh, uh, bh):
    global LAST_RESULT
    update = np.asarray(update)
    hidden = np.asarray(hidden)
    R = update.shape[0] // NCORES
    nc = _get_nc(R, 2)
    in_maps = make_in_maps(update, hidden, wz, uz, bz, wr, ur, br, wh, uh, bh)
    res = run_bass_kernel_spmd(nc, in_maps, list(range(NCORES)), trace=TRACE)
    LAST_RESULT = res
    out = np.empty((update.shape[0], H), np.float32)
    for i in range(NCORES):
        out[i * R:(i + 1) * R] = res.results[i]["outT"].T
    return out


# revision 27
# speedup vs baseline: 1.0158x; 1.0021x over previous
"""GRU cell kernel for Trainium2, data-parallel over 8 NeuronCores.

Math (per reference):
    z = sigmoid(x @ wz.T + h @ uz.T + bz)
    r = sigmoid(x @ wr.T + h @ ur.T + br)
    g = tanh(x @ wh.T + (r*h) @ uh.T + bh)
    out = (1-z)*h + z*g

Everything on-device is computed in TRANSPOSED layout ([feature, row]),
so that both matmul operands arrive with the contraction dim on
partitions without any on-device transpose.

Precision/speed split (validated against the fp32 reference offline;
device rel-err matches the numpy sim to 5 digits):
  - r-gate matmuls and (r*h)@uh: full fp8-e4m3 DoubleRow (2 contraction
    rows per PE cell per cycle -> ~2x bf16 throughput).
  - x@wz: K-rows 0..767 bf16, 768..1023 fp8 DR.
  - h@uz: K-rows 0..255 bf16, 256..1023 fp8 DR (the uz side tolerates
    far more fp8 than wz at the max-err metric).
  - x@wh: K-rows 0..511 bf16, 512..1023 fp8 DR.
  Sim rel-err 0.0185 vs gate 2e-2 (device matches the sim exactly).
All moving operands are pre-scaled x16 and all weights x128 on host, so
every PSUM holds 2048*(pre-activation); the activation instruction
undoes it with scale=1/2048 before bias.  (1-z) comes from a second
sigmoid with negated scale/bias on the otherwise-idle Scalar engine, so
(1-z)*h is precomputed in the z pass and the post-matmul tail chain is
just tanh -> z*g -> add.

DMA-issue notes: descriptor issue (DIRECT2D) costs ~650ns serialized on
the Sync queue (and ~1.6us on Scalar, where it also blocks activations,
so everything stays on Sync).  The layout minimizes descriptor count on
the critical path: fp8 weights are host-packed so a pair-tile is one
descriptor, x8/h8 load two k-blocks per descriptor via an AP rearrange,
and emission follows consumption order (r set, z set, xt/hbt, h set).
Chunk-1 operand loads are emitted before chunk-0's h-pass stores so the
stores' data-dependency waits cannot delay them in the queue.

Sharding: rows 16384 -> 8 cores x 2048 rows, weights replicated and
loaded once (reused across both row-chunks).
"""

import numpy as np
import ml_dtypes
from contextlib import ExitStack

import concourse.bass as bass
import concourse.bacc as bacc
import concourse.mybir as mybir
import concourse.tile as tile
from concourse.bass_utils import run_bass_kernel_spmd

H = 1024
N_ROWS = 16384
NCORES = 8
P = 128
KB = H // P            # 8 contraction blocks (bf16)
KP = KB // 2           # 4 fp8 DoubleRow contraction pairs
MB = H // P            # 8 output-feature blocks
NS = 512               # rows per matmul moving slice (one PSUM bank)
KZ8X = 1               # x@wz: trailing DR pairs in fp8 (of KP)
KZ8H = 3               # h@uz: trailing DR pairs in fp8 (asymmetric: the
                       # uz side tolerates more fp8 than wz — sim-verified
                       # at rel-err 0.0185 vs the 2e-2 gate)
KH8 = 2                # x@wh: trailing DR pairs in fp8 (of KP)
KBZX = KB - 2 * KZ8X   # 6 bf16 k-blocks in x@wz
KBZH = KB - 2 * KZ8H   # 4 bf16 k-blocks in h@uz
KBH = KB - 2 * KH8     # 4 bf16 k-blocks in x@wh

BF = mybir.dt.bfloat16
F8 = mybir.dt.float8e4
F32 = mybir.dt.float32
AF = mybir.ActivationFunctionType
DR = mybir.MatmulPerfMode.DoubleRow
bf16 = ml_dtypes.bfloat16
f8e4 = ml_dtypes.float8_e4m3

SX = 16.0              # moving-operand scale
SW = 128.0             # weight scale
INV_S = 1.0 / (SX * SW)

# Set by test harness to capture a trace; harness-facing default off.
TRACE = False
LAST_RESULT = None


def build_nc(R=N_ROWS // NCORES, CH=2):
    """Build the per-core Bass program. R rows per core, CH row-chunks."""
    RC = R // CH           # rows per chunk
    SL = RC // NS          # moving slices per chunk

    nc = bacc.Bacc(trn_type="TRN2", target_bir_lowering=False,
                   debug=False, enable_asserts=False)

    xb = nc.dram_tensor("xb", [H, R], BF, kind="ExternalInput").ap()
    x8 = nc.dram_tensor("x8", [H, R], F8, kind="ExternalInput").ap()
    hb = nc.dram_tensor("hb", [H, R], BF, kind="ExternalInput").ap()
    h8 = nc.dram_tensor("h8", [H, R], F8, kind="ExternalInput").ap()
    hc = nc.dram_tensor("hc", [H, R], BF, kind="ExternalInput").ap()
    wd = {
        nm: nc.dram_tensor(nm, [H, H], BF, kind="ExternalInput").ap()
        for nm in ("wzT", "uzT", "whT")
    }
    # fp8 weights host-packed as [P, KP, 2, H] so one pair-tile is one
    # contiguous-per-partition DMA descriptor
    wd8 = {
        nm: nc.dram_tensor(nm, [P, KP * 2 * H], F8, kind="ExternalInput").ap()
        for nm in ("wzT8", "uzT8", "whT8", "wrT8", "urT8", "uhT8")
    }
    bias = nc.dram_tensor("bias", [P, 4 * MB], F32, kind="ExternalInput").ap()
    outT = nc.dram_tensor("outT", [H, R], F32, kind="ExternalOutput").ap()

    with tile.TileContext(nc) as tc, ExitStack() as ctx:
        wbpool = ctx.enter_context(tc.tile_pool(name="wb", bufs=16))
        w8pool = ctx.enter_context(tc.tile_pool(name="w8", bufs=18))
        xpool = ctx.enter_context(tc.tile_pool(name="x", bufs=2))
        x8pool = ctx.enter_context(tc.tile_pool(name="x8", bufs=2))
        hbpool = ctx.enter_context(tc.tile_pool(name="hb", bufs=1))
        h8pool = ctx.enter_context(tc.tile_pool(name="h8", bufs=1))
        hcpool = ctx.enter_context(tc.tile_pool(name="hc", bufs=2))
        ctpool = ctx.enter_context(tc.tile_pool(name="ct", bufs=MB + 1))
        rh8pool = ctx.enter_context(tc.tile_pool(name="rh8", bufs=2))
        rpool = ctx.enter_context(tc.tile_pool(name="r", bufs=4))
        zpool = ctx.enter_context(tc.tile_pool(name="z", bufs=2 * MB))
        gpool = ctx.enter_context(tc.tile_pool(name="g", bufs=3))
        opool = ctx.enter_context(tc.tile_pool(name="o", bufs=4))
        cpool = ctx.enter_context(tc.tile_pool(name="c", bufs=1))
        pspool = ctx.enter_context(tc.tile_pool(name="ps", bufs=8, space="PSUM"))

        # Warm up the ACT table set (sigmoid_and_others covers tanh too) on an
        # instruction with minimal sync waits — walrus can't attach the
        # PSEUDO_LOAD_ACT_FUNC_SET to an activation that already carries two
        # sem waits ("Too many sync wait commands").
        warm = cpool.tile([P, 8], F32, tag="warm")
        nc.gpsimd.memset(warm[:], 0.0)
        nc.scalar.activation(warm[:], warm[:], AF.Sigmoid)

        # PE HAM warm-up: the PE clock gate defaults to 1.2GHz and only
        # reaches 2.4GHz after ~3.4us of sustained matmul activity, which
        # otherwise happens during the first real matmuls (~4us of cold
        # penalty measured).  Burn the otherwise-idle initial DMA window
        # with tiny scratch matmuls so the real work starts warm.
        scr = cpool.tile([P, P], BF, tag="scr")
        nc.gpsimd.memset(scr[:], 1.0)
        psd = pspool.tile([P, 64], F32, tag="ps", name="warmps")
        for _ in range(60):
            nc.tensor.matmul(psd[:], scr[:], scr[:, 0:64], start=True, stop=True)

        # bias column layout: [z:0..7 | r:8..15 | h:16..23 | -z:24..31]
        # (DMA emitted after the r set — first needed by the m0 sigmoid)
        bt = cpool.tile([P, 4 * MB], F32, tag="bias")
        GZ, GR, GH, GN = 0, 1, 2, 3

        def load_wb(name, nk):
            """nk leading bf16 k-tiles [P, H] of one weight matrix."""
            ts = []
            for k in range(nk):
                t = wbpool.tile([P, H], BF, tag="wb")
                nc.sync.dma_start(t[:], wd[name][k * P:(k + 1) * P, :])
                ts.append(t)
            return ts

        def load_w8(name, j0=0, j1=KP):
            """fp8 DR pair-tiles [P, 2, H], one descriptor each."""
            ts = []
            for j in range(j0, j1):
                t = w8pool.tile([P, 2, H], F8, tag="w8", name="t")
                nc.sync.dma_start(t[:, :, :],
                                  wd8[name][:, j * 2 * H:(j + 1) * 2 * H])
                ts.append(t)
            return ts

        def load_f8_ops(x8t, h8t, rows):
            """x8/h8 chunk loads, two k-blocks per descriptor."""
            for j in range(KP):
                ksl = slice(2 * j * P, (2 * j + 2) * P)
                nc.sync.dma_start(
                    x8t[:, 2 * j:2 * j + 2, :],
                    x8[ksl, rows].rearrange("(k p) c -> p k c", p=P))
                nc.sync.dma_start(
                    h8t[:, 2 * j:2 * j + 2, :],
                    h8[ksl, rows].rearrange("(k p) c -> p k c", p=P))

        def load_bf_ops(xt, hbt, rows):
            # hbt first: the r-pass epilogue (rh = r*16h) consumes hbt
            # m-block by m-block well before the z pass needs xt
            for k in range(KB):
                ksl = slice(k * P, (k + 1) * P)
                nc.sync.dma_start(hbt[:, k * RC:(k + 1) * RC], hb[ksl, rows])
            for k in range(KBZX):
                ksl = slice(k * P, (k + 1) * P)
                nc.sync.dma_start(xt[:, k * RC:(k + 1) * RC], xb[ksl, rows])

        # ---- chunk-0 r set (weights interleaved with operands), then z
        # set, then xt/hbt, then h set — matching consumption order ----
        x8t0 = x8pool.tile([P, KB, RC], F8, tag="x8", name="x8t")
        h8t0 = h8pool.tile([P, KB, RC], F8, tag="h8", name="h8t")
        wr, ur = [], []
        for j in range(KP):
            if j == 0:
                # first tiles split in half so the very first matmul's
                # dependencies land sooner
                t = w8pool.tile([P, 2, H], F8, tag="w8", name="t")
                nc.sync.dma_start(t[:, 0, :], wd8["wrT8"][:, 0:H])
                nc.sync.dma_start(x8t0[:, 0, :], x8[0:P, 0:RC])
                nc.sync.dma_start(t[:, 1, :], wd8["wrT8"][:, H:2 * H])
                nc.sync.dma_start(x8t0[:, 1, :], x8[P:2 * P, 0:RC])
                wr.append(t)
            else:
                wr += load_w8("wrT8", j, j + 1)
            ksl = slice(2 * j * P, (2 * j + 2) * P)
            if j != 0:
                nc.sync.dma_start(
                    x8t0[:, 2 * j:2 * j + 2, :],
                    x8[ksl, 0:RC].rearrange("(k p) c -> p k c", p=P))
            ur += load_w8("urT8", j, j + 1)
            if j == 0:
                for k in range(2):
                    ks1 = slice(k * P, (k + 1) * P)
                    nc.sync.dma_start(h8t0[:, k, :], h8[ks1, 0:RC])
            else:
                nc.sync.dma_start(
                    h8t0[:, 2 * j:2 * j + 2, :],
                    h8[ksl, 0:RC].rearrange("(k p) c -> p k c", p=P))
        nc.sync.dma_start(bt[:], bias[:])

        # hbt before the z weights: the r-pass epilogue (rh = r*16h)
        # consumes hbt m-block by m-block starting ~halfway into the r pass
        hbt0 = hbpool.tile([P, KB * RC], BF, tag="hb")
        for k in range(KB):
            ksl = slice(k * P, (k + 1) * P)
            nc.sync.dma_start(hbt0[:, k * RC:(k + 1) * RC], hb[ksl, 0:RC])

        wz = load_wb("wzT", KBZX)
        wz8 = load_w8("wzT8", KP - KZ8X, KP)
        uz = load_wb("uzT", KBZH)
        uz8 = load_w8("uzT8", KP - KZ8H, KP)

        xt0 = xpool.tile([P, KBZX * RC], BF, tag="x")
        for k in range(KBZX):
            ksl = slice(k * P, (k + 1) * P)
            nc.sync.dma_start(xt0[:, k * RC:(k + 1) * RC], xb[ksl, 0:RC])

        wh = load_wb("whT", KBH)
        wh8 = load_w8("whT8", KP - KH8, KP)
        uh = load_w8("uhT8")

        for c in range(CH):
            rows = slice(c * RC, (c + 1) * RC)

            if c == 0:
                x8t, h8t, xt, hbt = x8t0, h8t0, xt0, hbt0

            # ---- r pass (all fp8 DoubleRow) ----
            rh8 = rh8pool.tile([P, MB, RC], F8, tag="rh8")
            for m in range(MB):
                msl = slice(m * P, (m + 1) * P)
                ps = [pspool.tile([P, NS], F32, tag="ps", name="ps") for _ in range(SL)]
                for j in range(KP):
                    for s in range(SL):
                        nc.tensor.matmul(
                            ps[s][:], wr[j][:, :, msl],
                            x8t[:, 2 * j:2 * j + 2, s * NS:(s + 1) * NS],
                            start=(j == 0), stop=False, perf_mode=DR)
                    for s in range(SL):
                        nc.tensor.matmul(
                            ps[s][:], ur[j][:, :, msl],
                            h8t[:, 2 * j:2 * j + 2, s * NS:(s + 1) * NS],
                            start=False, stop=(j == KP - 1), perf_mode=DR)
                for s in range(SL):
                    rt = rpool.tile([P, NS], BF, tag="r")
                    nc.scalar.activation(rt[:], ps[s][:], AF.Sigmoid,
                                         bias=bt[:, GR * MB + m: GR * MB + m + 1],
                                         scale=INV_S)
                    # rh8 = e4m3(r * 16h): hbt is 16h, rt unscaled in (0,1)
                    nc.vector.tensor_mul(
                        rh8[:, m, s * NS:(s + 1) * NS], rt[:],
                        hbt[:, m * RC + s * NS: m * RC + (s + 1) * NS])

            # ---- z pass (bf16 k0..5 + fp8 DR pair k6,7) ----
            # also computes ct = (1-z)*h via a second sigmoid with negated
            # scale/bias, so the h~ pass tail is just tanh -> z*g -> +ct
            zts, cts = [], []
            for m in range(MB):
                msl = slice(m * P, (m + 1) * P)
                hct = hcpool.tile([P, RC], BF, tag="hc")
                nc.sync.dma_start(hct[:], hc[msl, rows])
                ps = [pspool.tile([P, NS], F32, tag="ps", name="ps") for _ in range(SL)]
                for k in range(KBZX):
                    for s in range(SL):
                        nc.tensor.matmul(
                            ps[s][:], wz[k][:, msl],
                            xt[:, k * RC + s * NS: k * RC + (s + 1) * NS],
                            start=(k == 0), stop=False)
                for jj, j in enumerate(range(KP - KZ8X, KP)):
                    for s in range(SL):
                        nc.tensor.matmul(
                            ps[s][:], wz8[jj][:, :, msl],
                            x8t[:, 2 * j:2 * j + 2, s * NS:(s + 1) * NS],
                            start=False, stop=False, perf_mode=DR)
                for k in range(KBZH):
                    for s in range(SL):
                        nc.tensor.matmul(
                            ps[s][:], uz[k][:, msl],
                            hbt[:, k * RC + s * NS: k * RC + (s + 1) * NS],
                            start=False, stop=False)
                for jj, j in enumerate(range(KP - KZ8H, KP)):
                    for s in range(SL):
                        nc.tensor.matmul(
                            ps[s][:], uz8[jj][:, :, msl],
                            h8t[:, 2 * j:2 * j + 2, s * NS:(s + 1) * NS],
                            start=False, stop=(j == KP - 1), perf_mode=DR)
                zm = []
                ctt = ctpool.tile([P, RC], BF, tag="ct")
                for s in range(SL):
                    ssl = slice(s * NS, (s + 1) * NS)
                    zt = zpool.tile([P, NS], BF, tag="z")
                    nc.scalar.activation(zt[:], ps[s][:], AF.Sigmoid,
                                         bias=bt[:, GZ * MB + m: GZ * MB + m + 1],
                                         scale=INV_S)
                    zm.append(zt)
                    zct = rpool.tile([P, NS], BF, tag="r")
                    nc.scalar.activation(zct[:], ps[s][:], AF.Sigmoid,
                                         bias=bt[:, GN * MB + m: GN * MB + m + 1],
                                         scale=-INV_S)
                    nc.vector.tensor_mul(ctt[:, ssl], zct[:], hct[:, ssl])
                zts.append(zm)
                cts.append(ctt)

            # chunk c+1 operand loads, emitted BEFORE this chunk's h-pass
            # stores so the stores' data waits can't delay them in the
            # Sync queue
            if c + 1 < CH:
                nrows = slice((c + 1) * RC, (c + 2) * RC)
                nx8t = x8pool.tile([P, KB, RC], F8, tag="x8", name="x8t")
                nh8t = h8pool.tile([P, KB, RC], F8, tag="h8", name="h8t")
                load_f8_ops(nx8t, nh8t, nrows)
                nxt = xpool.tile([P, KBZX * RC], BF, tag="x")
                nhbt = hbpool.tile([P, KB * RC], BF, tag="hb")
                load_bf_ops(nxt, nhbt, nrows)

            # ---- h~ pass (x@wh bf16 k0..3 + fp8 DR pairs; (r*h)@uh fp8 DR)
            #      + combine ----
            for m in range(MB):
                msl = slice(m * P, (m + 1) * P)
                ps = [pspool.tile([P, NS], F32, tag="ps", name="ps") for _ in range(SL)]
                for k in range(KBH):
                    for s in range(SL):
                        nc.tensor.matmul(
                            ps[s][:], wh[k][:, msl],
                            xt[:, k * RC + s * NS: k * RC + (s + 1) * NS],
                            start=(k == 0), stop=False)
                for jj, j in enumerate(range(KP - KH8, KP)):
                    for s in range(SL):
                        nc.tensor.matmul(
                            ps[s][:], wh8[jj][:, :, msl],
                            x8t[:, 2 * j:2 * j + 2, s * NS:(s + 1) * NS],
                            start=False, stop=False, perf_mode=DR)
                for j in range(KP):
                    for s in range(SL):
                        nc.tensor.matmul(
                            ps[s][:], uh[j][:, :, msl],
                            rh8[:, 2 * j:2 * j + 2, s * NS:(s + 1) * NS],
                            start=False, stop=(j == KP - 1), perf_mode=DR)
                # the very last m-block runs its epilogue at 256-col
                # granularity so the post-matmul drain pipeline is shorter
                fine = (c == CH - 1 and m == MB - 1)
                NE = NS // 2 if fine else NS
                for s in range(SL):
                    for e in range(NS // NE):
                        esl = slice(s * NS + e * NE, s * NS + (e + 1) * NE)
                        pesl = slice(e * NE, (e + 1) * NE)
                        gt = gpool.tile([P, NE], F32, tag="g")
                        nc.scalar.activation(gt[:], ps[s][:, pesl], AF.Tanh,
                                             bias=bt[:, GH * MB + m:
                                                     GH * MB + m + 1],
                                             scale=INV_S)
                        # z*g ; (1-z)*h + z*g
                        nc.vector.tensor_mul(gt[:], zts[m][s][:, pesl], gt[:])
                        ot = opool.tile([P, NE], F32, tag="o")
                        nc.vector.tensor_add(ot[:], gt[:], cts[m][:, esl])
                        # one store descriptor per piece: descriptor issue
                        # (~650ns, serialized on Sync) costs more than the
                        # extra per-ring transfer time
                        nc.sync.dma_start(outT[msl, c * RC + s * NS + e * NE:
                                               c * RC + s * NS + (e + 1) * NE],
                                          ot[:])

            if c + 1 < CH:
                x8t, h8t, xt, hbt = nx8t, nh8t, nxt, nhbt

    nc.compile()
    return nc


_NC_CACHE = {}


def _get_nc(R, CH):
    key = (R, CH)
    if key not in _NC_CACHE:
        _NC_CACHE[key] = build_nc(R, CH)
    return _NC_CACHE[key]


def make_in_maps(update, hidden, wz, uz, bz, wr, ur, br, wh, uh, bh,
                 ncores=NCORES):
    wmap = {}
    for nm, w in (("wzT", wz), ("uzT", uz), ("whT", wh)):
        ws = np.ascontiguousarray(np.asarray(w, np.float32).T * SW)
        wmap[nm] = ws.astype(bf16)
    # fp8 weights packed [P, KP, 2, H]: pack[p, j, i, m] = W.T[(2j+i)*P+p, m]
    for nm, w in (("wzT8", wz), ("uzT8", uz), ("whT8", wh),
                  ("wrT8", wr), ("urT8", ur), ("uhT8", uh)):
        ws = (np.asarray(w, np.float32).T * SW).astype(f8e4)
        pk = ws.reshape(KP, 2, P, H).transpose(2, 0, 1, 3).reshape(P, KP * 2 * H)
        wmap[nm] = np.ascontiguousarray(pk)
    bias = np.empty((P, 4 * MB), np.float32)
    for g, b in enumerate((bz, br, bh)):
        bias[:, g * MB:(g + 1) * MB] = np.asarray(b, np.float32).reshape(MB, P).T
    bias[:, 3 * MB:4 * MB] = -bias[:, 0:MB]
    rows = update.shape[0]
    rc = rows // ncores
    in_maps = []
    for i in range(ncores):
        sl = slice(i * rc, (i + 1) * rc)
        xTs = np.ascontiguousarray(np.asarray(update[sl], np.float32).T)
        hTs = np.ascontiguousarray(np.asarray(hidden[sl], np.float32).T)
        x16 = xTs * SX
        h16 = hTs * SX
        in_maps.append(dict(
            xb=x16.astype(bf16), x8=x16.astype(f8e4),
            hb=h16.astype(bf16), h8=h16.astype(f8e4),
            hc=hTs.astype(bf16), bias=bias, **wmap))
    return in_maps


def kernel(update, hidden, wz, uz, bz, wr, ur, br, wh, uh, bh):
    global LAST_RESULT
    update = np.asarray(update)
    hidden = np.asarray(hidden)
    R = update.shape[0] // NCORES
    nc = _get_nc(R, 2)
    in_maps = make_in_maps(update, hidden, wz, uz, bz, wr, ur, br, wh, uh, bh)
    res = run_bass_kernel_spmd(nc, in_maps, list(range(NCORES)), trace=TRACE)
    LAST_RESULT = res
    out = np.empty((update.shape[0], H), np.float32)
    for i in range(NCORES):
        out[i * R:(i + 1) * R] = res.results[i]["outT"].T
    return out


# revision 28
# speedup vs baseline: 1.0229x; 1.0070x over previous
"""GRU cell kernel for Trainium2, data-parallel over 8 NeuronCores.

Math (per reference):
    z = sigmoid(x @ wz.T + h @ uz.T + bz)
    r = sigmoid(x @ wr.T + h @ ur.T + br)
    g = tanh(x @ wh.T + (r*h) @ uh.T + bh)
    out = (1-z)*h + z*g

Everything on-device is computed in TRANSPOSED layout ([feature, row]),
so that both matmul operands arrive with the contraction dim on
partitions without any on-device transpose.

Precision/speed split (validated against the fp32 reference offline;
device rel-err matches the numpy sim to 5 digits):
  - r-gate matmuls and (r*h)@uh: full fp8-e4m3 DoubleRow (2 contraction
    rows per PE cell per cycle -> ~2x bf16 throughput).
  - x@wz: K-rows 0..767 bf16, 768..1023 fp8 DR.
  - h@uz: K-rows 0..255 bf16, 256..1023 fp8 DR (the uz side tolerates
    far more fp8 than wz at the max-err metric).
  - x@wh: K-rows 0..511 bf16, 512..1023 fp8 DR.
  Sim rel-err 0.0185 vs gate 2e-2 (device matches the sim exactly).
All moving operands are pre-scaled x16 and all weights x128 on host, so
every PSUM holds 2048*(pre-activation); the activation instruction
undoes it with scale=1/2048 before bias.  (1-z) comes from a second
sigmoid with negated scale/bias on the otherwise-idle Scalar engine, so
(1-z)*h is precomputed in the z pass and the post-matmul tail chain is
just tanh -> z*g -> add.

DMA-issue notes: descriptor issue (DIRECT2D) costs ~650ns serialized on
the Sync queue (and ~1.6us on Scalar, where it also blocks activations,
so everything stays on Sync).  The layout minimizes descriptor count on
the critical path: fp8 weights are host-packed so a pair-tile is one
descriptor, x8/h8 load two k-blocks per descriptor via an AP rearrange,
and emission follows consumption order (r set, z set, xt/hbt, h set).
Chunk-1 operand loads are emitted before chunk-0's h-pass stores so the
stores' data-dependency waits cannot delay them in the queue.

Sharding: rows 16384 -> 8 cores x 2048 rows, weights replicated and
loaded once (reused across both row-chunks).
"""

import numpy as np
import ml_dtypes
from contextlib import ExitStack

import concourse.bass as bass
import concourse.bacc as bacc
import concourse.mybir as mybir
import concourse.tile as tile
from concourse.bass_utils import run_bass_kernel_spmd

H = 1024
N_ROWS = 16384
NCORES = 8
P = 128
KB = H // P            # 8 contraction blocks (bf16)
KP = KB // 2           # 4 fp8 DoubleRow contraction pairs
MB = H // P            # 8 output-feature blocks
NS = 512               # rows per matmul moving slice (one PSUM bank)
KZ8X = 1               # x@wz: trailing DR pairs in fp8 (of KP)
KZ8H = 3               # h@uz: trailing DR pairs in fp8 (asymmetric: the
                       # uz side tolerates more fp8 than wz — sim-verified
                       # at rel-err 0.0185 vs the 2e-2 gate)
KH8 = 2                # x@wh: trailing DR pairs in fp8 (of KP)
KBZX = KB - 2 * KZ8X   # 6 bf16 k-blocks in x@wz
KBZH = KB - 2 * KZ8H   # 4 bf16 k-blocks in h@uz
KBH = KB - 2 * KH8     # 4 bf16 k-blocks in x@wh

BF = mybir.dt.bfloat16
F8 = mybir.dt.float8e4
F32 = mybir.dt.float32
AF = mybir.ActivationFunctionType
DR = mybir.MatmulPerfMode.DoubleRow
bf16 = ml_dtypes.bfloat16
f8e4 = ml_dtypes.float8_e4m3

SX = 16.0              # moving-operand scale
SW = 128.0             # weight scale
INV_S = 1.0 / (SX * SW)

# Set by test harness to capture a trace; harness-facing default off.
TRACE = False
LAST_RESULT = None


def build_nc(R=N_ROWS // NCORES, CH=2):
    """Build the per-core Bass program. R rows per core, CH row-chunks."""
    RC = R // CH           # rows per chunk
    SL = RC // NS          # moving slices per chunk

    nc = bacc.Bacc(trn_type="TRN2", target_bir_lowering=False,
                   debug=False, enable_asserts=False)

    xb = nc.dram_tensor("xb", [H, R], BF, kind="ExternalInput").ap()
    x8 = nc.dram_tensor("x8", [H, R], F8, kind="ExternalInput").ap()
    hb = nc.dram_tensor("hb", [H, R], BF, kind="ExternalInput").ap()
    h8 = nc.dram_tensor("h8", [H, R], F8, kind="ExternalInput").ap()
    hc = nc.dram_tensor("hc", [H, R], BF, kind="ExternalInput").ap()
    wd = {
        nm: nc.dram_tensor(nm, [H, H], BF, kind="ExternalInput").ap()
        for nm in ("wzT", "uzT", "whT")
    }
    # fp8 weights host-packed as [P, KP, 2, H] so one pair-tile is one
    # contiguous-per-partition DMA descriptor
    wd8 = {
        nm: nc.dram_tensor(nm, [P, KP * 2 * H], F8, kind="ExternalInput").ap()
        for nm in ("wzT8", "uzT8", "whT8", "wrT8", "urT8", "uhT8")
    }
    bias = nc.dram_tensor("bias", [P, 4 * MB], F32, kind="ExternalInput").ap()
    outT = nc.dram_tensor("outT", [H, R], F32, kind="ExternalOutput").ap()

    with tile.TileContext(nc) as tc, ExitStack() as ctx:
        wbpool = ctx.enter_context(tc.tile_pool(name="wb", bufs=16))
        w8pool = ctx.enter_context(tc.tile_pool(name="w8", bufs=18))
        xpool = ctx.enter_context(tc.tile_pool(name="x", bufs=2))
        x8pool = ctx.enter_context(tc.tile_pool(name="x8", bufs=2))
        hbpool = ctx.enter_context(tc.tile_pool(name="hb", bufs=1))
        h8pool = ctx.enter_context(tc.tile_pool(name="h8", bufs=1))
        hcpool = ctx.enter_context(tc.tile_pool(name="hc", bufs=2))
        ctpool = ctx.enter_context(tc.tile_pool(name="ct", bufs=MB + 1))
        rh8pool = ctx.enter_context(tc.tile_pool(name="rh8", bufs=2))
        rpool = ctx.enter_context(tc.tile_pool(name="r", bufs=4))
        zpool = ctx.enter_context(tc.tile_pool(name="z", bufs=2 * MB))
        gpool = ctx.enter_context(tc.tile_pool(name="g", bufs=3))
        opool = ctx.enter_context(tc.tile_pool(name="o", bufs=4))
        cpool = ctx.enter_context(tc.tile_pool(name="c", bufs=1))
        pspool = ctx.enter_context(tc.tile_pool(name="ps", bufs=8, space="PSUM"))

        # Warm up the ACT table set (sigmoid_and_others covers tanh too) on an
        # instruction with minimal sync waits — walrus can't attach the
        # PSEUDO_LOAD_ACT_FUNC_SET to an activation that already carries two
        # sem waits ("Too many sync wait commands").
        warm = cpool.tile([P, 8], F32, tag="warm")
        nc.gpsimd.memset(warm[:], 0.0)
        nc.scalar.activation(warm[:], warm[:], AF.Sigmoid)

        # bias column layout: [z:0..7 | r:8..15 | h:16..23 | -z:24..31]
        # (DMA emitted after the r set — first needed by the m0 sigmoid)
        bt = cpool.tile([P, 4 * MB], F32, tag="bias")
        GZ, GR, GH, GN = 0, 1, 2, 3

        def load_wb(name, nk):
            """nk leading bf16 k-tiles [P, H] of one weight matrix."""
            ts = []
            for k in range(nk):
                t = wbpool.tile([P, H], BF, tag="wb")
                nc.sync.dma_start(t[:], wd[name][k * P:(k + 1) * P, :])
                ts.append(t)
            return ts

        def load_w8(name, j0=0, j1=KP):
            """fp8 DR pair-tiles [P, 2, H], one descriptor each."""
            ts = []
            for j in range(j0, j1):
                t = w8pool.tile([P, 2, H], F8, tag="w8", name="t")
                nc.sync.dma_start(t[:, :, :],
                                  wd8[name][:, j * 2 * H:(j + 1) * 2 * H])
                ts.append(t)
            return ts

        def load_f8_ops(x8t, h8t, rows):
            """x8/h8 chunk loads, two k-blocks per descriptor."""
            for j in range(KP):
                ksl = slice(2 * j * P, (2 * j + 2) * P)
                nc.sync.dma_start(
                    x8t[:, 2 * j:2 * j + 2, :],
                    x8[ksl, rows].rearrange("(k p) c -> p k c", p=P))
                nc.sync.dma_start(
                    h8t[:, 2 * j:2 * j + 2, :],
                    h8[ksl, rows].rearrange("(k p) c -> p k c", p=P))

        def load_bf_ops(xt, hbt, rows):
            # hbt first: the r-pass epilogue (rh = r*16h) consumes hbt
            # m-block by m-block well before the z pass needs xt
            for k in range(KB):
                ksl = slice(k * P, (k + 1) * P)
                nc.sync.dma_start(hbt[:, k * RC:(k + 1) * RC], hb[ksl, rows])
            for k in range(KBZX):
                ksl = slice(k * P, (k + 1) * P)
                nc.sync.dma_start(xt[:, k * RC:(k + 1) * RC], xb[ksl, rows])

        # ---- chunk-0 r set (weights interleaved with operands), then z
        # set, then xt/hbt, then h set — matching consumption order ----
        x8t0 = x8pool.tile([P, KB, RC], F8, tag="x8", name="x8t")
        h8t0 = h8pool.tile([P, KB, RC], F8, tag="h8", name="h8t")
        wr, ur = [], []
        for j in range(KP):
            if j == 0:
                # first tiles split in half so the very first matmul's
                # dependencies land sooner
                t = w8pool.tile([P, 2, H], F8, tag="w8", name="t")
                nc.sync.dma_start(t[:, 0, :], wd8["wrT8"][:, 0:H])
                nc.sync.dma_start(x8t0[:, 0, :], x8[0:P, 0:RC])
                nc.sync.dma_start(t[:, 1, :], wd8["wrT8"][:, H:2 * H])
                nc.sync.dma_start(x8t0[:, 1, :], x8[P:2 * P, 0:RC])
                wr.append(t)
            else:
                wr += load_w8("wrT8", j, j + 1)
            ksl = slice(2 * j * P, (2 * j + 2) * P)
            if j != 0:
                nc.sync.dma_start(
                    x8t0[:, 2 * j:2 * j + 2, :],
                    x8[ksl, 0:RC].rearrange("(k p) c -> p k c", p=P))
            ur += load_w8("urT8", j, j + 1)
            if j == 0:
                for k in range(2):
                    ks1 = slice(k * P, (k + 1) * P)
                    nc.sync.dma_start(h8t0[:, k, :], h8[ks1, 0:RC])
            else:
                nc.sync.dma_start(
                    h8t0[:, 2 * j:2 * j + 2, :],
                    h8[ksl, 0:RC].rearrange("(k p) c -> p k c", p=P))
        nc.sync.dma_start(bt[:], bias[:])

        # hbt before the z weights: the r-pass epilogue (rh = r*16h)
        # consumes hbt m-block by m-block starting ~halfway into the r pass
        hbt0 = hbpool.tile([P, KB * RC], BF, tag="hb")
        for k in range(KB):
            ksl = slice(k * P, (k + 1) * P)
            nc.sync.dma_start(hbt0[:, k * RC:(k + 1) * RC], hb[ksl, 0:RC])

        wz = load_wb("wzT", KBZX)
        wz8 = load_w8("wzT8", KP - KZ8X, KP)
        uz = load_wb("uzT", KBZH)
        uz8 = load_w8("uzT8", KP - KZ8H, KP)

        xt0 = xpool.tile([P, KBZX * RC], BF, tag="x")
        for k in range(KBZX):
            ksl = slice(k * P, (k + 1) * P)
            nc.sync.dma_start(xt0[:, k * RC:(k + 1) * RC], xb[ksl, 0:RC])

        wh = load_wb("whT", KBH)
        wh8 = load_w8("whT8", KP - KH8, KP)
        uh = load_w8("uhT8")

        for c in range(CH):
            rows = slice(c * RC, (c + 1) * RC)

            if c == 0:
                x8t, h8t, xt, hbt = x8t0, h8t0, xt0, hbt0

            # ---- r pass (all fp8 DoubleRow) ----
            rh8 = rh8pool.tile([P, MB, RC], F8, tag="rh8")
            for m in range(MB):
                msl = slice(m * P, (m + 1) * P)
                ps = [pspool.tile([P, NS], F32, tag="ps", name="ps") for _ in range(SL)]
                for j in range(KP):
                    for s in range(SL):
                        nc.tensor.matmul(
                            ps[s][:], wr[j][:, :, msl],
                            x8t[:, 2 * j:2 * j + 2, s * NS:(s + 1) * NS],
                            start=(j == 0), stop=False, perf_mode=DR)
                    for s in range(SL):
                        nc.tensor.matmul(
                            ps[s][:], ur[j][:, :, msl],
                            h8t[:, 2 * j:2 * j + 2, s * NS:(s + 1) * NS],
                            start=False, stop=(j == KP - 1), perf_mode=DR)
                for s in range(SL):
                    rt = rpool.tile([P, NS], BF, tag="r")
                    nc.scalar.activation(rt[:], ps[s][:], AF.Sigmoid,
                                         bias=bt[:, GR * MB + m: GR * MB + m + 1],
                                         scale=INV_S)
                    # rh8 = e4m3(r * 16h): hbt is 16h, rt unscaled in (0,1)
                    nc.vector.tensor_mul(
                        rh8[:, m, s * NS:(s + 1) * NS], rt[:],
                        hbt[:, m * RC + s * NS: m * RC + (s + 1) * NS])

            # ---- z pass (bf16 k0..5 + fp8 DR pair k6,7) ----
            # also computes ct = (1-z)*h via a second sigmoid with negated
            # scale/bias, so the h~ pass tail is just tanh -> z*g -> +ct
            zts, cts = [], []
            for m in range(MB):
                msl = slice(m * P, (m + 1) * P)
                hct = hcpool.tile([P, RC], BF, tag="hc")
                nc.sync.dma_start(hct[:], hc[msl, rows])
                ps = [pspool.tile([P, NS], F32, tag="ps", name="ps") for _ in range(SL)]
                for k in range(KBZX):
                    for s in range(SL):
                        nc.tensor.matmul(
                            ps[s][:], wz[k][:, msl],
                            xt[:, k * RC + s * NS: k * RC + (s + 1) * NS],
                            start=(k == 0), stop=False)
                for jj, j in enumerate(range(KP - KZ8X, KP)):
                    for s in range(SL):
                        nc.tensor.matmul(
                            ps[s][:], wz8[jj][:, :, msl],
                            x8t[:, 2 * j:2 * j + 2, s * NS:(s + 1) * NS],
                            start=False, stop=False, perf_mode=DR)
                for k in range(KBZH):
                    for s in range(SL):
                        nc.tensor.matmul(
                            ps[s][:], uz[k][:, msl],
                            hbt[:, k * RC + s * NS: k * RC + (s + 1) * NS],
                            start=False, stop=False)
                for jj, j in enumerate(range(KP - KZ8H, KP)):
                    for s in range(SL):
                        nc.tensor.matmul(
                            ps[s][:], uz8[jj][:, :, msl],
                            h8t[:, 2 * j:2 * j + 2, s * NS:(s + 1) * NS],
                            start=False, stop=(j == KP - 1), perf_mode=DR)
                zm = []
                ctt = ctpool.tile([P, RC], BF, tag="ct")
                for s in range(SL):
                    ssl = slice(s * NS, (s + 1) * NS)
                    zt = zpool.tile([P, NS], BF, tag="z")
                    nc.scalar.activation(zt[:], ps[s][:], AF.Sigmoid,
                                         bias=bt[:, GZ * MB + m: GZ * MB + m + 1],
                                         scale=INV_S)
                    zm.append(zt)
                    zct = rpool.tile([P, NS], BF, tag="r")
                    nc.scalar.activation(zct[:], ps[s][:], AF.Sigmoid,
                                         bias=bt[:, GN * MB + m: GN * MB + m + 1],
                                         scale=-INV_S)
                    nc.vector.tensor_mul(ctt[:, ssl], zct[:], hct[:, ssl])
                zts.append(zm)
                cts.append(ctt)

            # chunk c+1 operand loads, emitted BEFORE this chunk's h-pass
            # stores so the stores' data waits can't delay them in the
            # Sync queue
            if c + 1 < CH:
                nrows = slice((c + 1) * RC, (c + 2) * RC)
                nx8t = x8pool.tile([P, KB, RC], F8, tag="x8", name="x8t")
                nh8t = h8pool.tile([P, KB, RC], F8, tag="h8", name="h8t")
                load_f8_ops(nx8t, nh8t, nrows)
                nxt = xpool.tile([P, KBZX * RC], BF, tag="x")
                nhbt = hbpool.tile([P, KB * RC], BF, tag="hb")
                load_bf_ops(nxt, nhbt, nrows)

            # ---- h~ pass (x@wh bf16 k0..3 + fp8 DR pairs; (r*h)@uh fp8 DR)
            #      + combine ----
            for m in range(MB):
                msl = slice(m * P, (m + 1) * P)
                ps = [pspool.tile([P, NS], F32, tag="ps", name="ps") for _ in range(SL)]
                for k in range(KBH):
                    for s in range(SL):
                        nc.tensor.matmul(
                            ps[s][:], wh[k][:, msl],
                            xt[:, k * RC + s * NS: k * RC + (s + 1) * NS],
                            start=(k == 0), stop=False)
                for jj, j in enumerate(range(KP - KH8, KP)):
                    for s in range(SL):
                        nc.tensor.matmul(
                            ps[s][:], wh8[jj][:, :, msl],
                            x8t[:, 2 * j:2 * j + 2, s * NS:(s + 1) * NS],
                            start=False, stop=False, perf_mode=DR)
                for j in range(KP):
                    for s in range(SL):
                        nc.tensor.matmul(
                            ps[s][:], uh[j][:, :, msl],
                            rh8[:, 2 * j:2 * j + 2, s * NS:(s + 1) * NS],
                            start=False, stop=(j == KP - 1), perf_mode=DR)
                # the very last m-block runs its epilogue at 256-col
                # granularity so the post-matmul drain pipeline is shorter
                fine = (c == CH - 1 and m == MB - 1)
                NE = NS // 2 if fine else NS
                for s in range(SL):
                    for e in range(NS // NE):
                        esl = slice(s * NS + e * NE, s * NS + (e + 1) * NE)
                        pesl = slice(e * NE, (e + 1) * NE)
                        gt = gpool.tile([P, NE], F32, tag="g")
                        nc.scalar.activation(gt[:], ps[s][:, pesl], AF.Tanh,
                                             bias=bt[:, GH * MB + m:
                                                     GH * MB + m + 1],
                                             scale=INV_S)
                        # z*g ; (1-z)*h + z*g
                        nc.vector.tensor_mul(gt[:], zts[m][s][:, pesl], gt[:])
                        ot = opool.tile([P, NE], F32, tag="o")
                        nc.vector.tensor_add(ot[:], gt[:], cts[m][:, esl])
                        # one store descriptor per piece: descriptor issue
                        # (~650ns, serialized on Sync) costs more than the
                        # extra per-ring transfer time
                        nc.sync.dma_start(outT[msl, c * RC + s * NS + e * NE:
                                               c * RC + s * NS + (e + 1) * NE],
                                          ot[:])

            if c + 1 < CH:
                x8t, h8t, xt, hbt = nx8t, nh8t, nxt, nhbt

    nc.compile()
    return nc


_NC_CACHE = {}


def _get_nc(R, CH):
    key = (R, CH)
    if key not in _NC_CACHE:
        _NC_CACHE[key] = build_nc(R, CH)
    return _NC_CACHE[key]


def make_in_maps(update, hidden, wz, uz, bz, wr, ur, br, wh, uh, bh,
                 ncores=NCORES):
    wmap = {}
    for nm, w in (("wzT", wz), ("uzT", uz), ("whT", wh)):
        ws = np.ascontiguousarray(np.asarray(w, np.float32).T * SW)
        wmap[nm] = ws.astype(bf16)
    # fp8 weights packed [P, KP, 2, H]: pack[p, j, i, m] = W.T[(2j+i)*P+p, m]
    for nm, w in (("wzT8", wz), ("uzT8", uz), ("whT8", wh),
                  ("wrT8", wr), ("urT8", ur), ("uhT8", uh)):
        ws = (np.asarray(w, np.float32).T * SW).astype(f8e4)
        pk = ws.reshape(KP, 2, P, H).transpose(2, 0, 1, 3).reshape(P, KP * 2 * H)
        wmap[nm] = np.ascontiguousarray(pk)
    bias = np.empty((P, 4 * MB), np.float32)
    for g, b in enumerate((bz, br, bh)):
        bias[:, g * MB:(g + 1) * MB] = np.asarray(b, np.float32).reshape(MB, P).T
    bias[:, 3 * MB:4 * MB] = -bias[:, 0:MB]
    rows = update.shape[0]
    rc = rows // ncores
    in_maps = []
    for i in range(ncores):
        sl = slice(i * rc, (i + 1) * rc)
        xTs = np.ascontiguousarray(np.asarray(update[sl], np.float32).T)
        hTs = np.ascontiguousarray(np.asarray(hidden[sl], np.float32).T)
        x16 = xTs * SX
        h16 = hTs * SX
        in_maps.append(dict(
            xb=x16.astype(bf16), x8=x16.astype(f8e4),
            hb=h16.astype(bf16), h8=h16.astype(f8e4),
            hc=hTs.astype(bf16), bias=bias, **wmap))
    return in_maps


def kernel(update, hidden, wz, uz, bz, wr, ur, br, wh, uh, bh):
    global LAST_RESULT
    update = np.asarray(update)
    hidden = np.asarray(hidden)
    R = update.shape[0] // NCORES
    nc = _get_nc(R, 2)
    in_maps = make_in_maps(update, hidden, wz, uz, bz, wr, ur, br, wh, uh, bh)
    res = run_bass_kernel_spmd(nc, in_maps, list(range(NCORES)), trace=TRACE)
    LAST_RESULT = res
    out = np.empty((update.shape[0], H), np.float32)
    for i in range(NCORES):
        out[i * R:(i + 1) * R] = res.results[i]["outT"].T
    return out
